# revision 1
# baseline (speedup 1.0000x reference)
"""Bass/Trainium2 kernel for nn_Expert_WNO2d (8-expert gated WaveConv2d mixture).

Math: the reference is linear in x. Every expert passes the fine Haar detail
levels (1..3) through unchanged and only channel-mixes the coarsest (level-4)
approximation + detail coefficients. With gate slots s weighting experts
PERM = (0,1,2,3,4,5,4,5), the output collapses to

    y[b] = G[b] * x[b] + rep8( adj[b] )                      (rep8 = 8x8 block broadcast)
    adj[b] = 0.125 * idwt4( sum_e geff[b,e] * (W_e . c4[b]) ) - (G[b]/64) * s8[b]

where s8 = 8x8 block sums of x, c4 = level-4 Haar coefficients (from s8),
G[b] = sum_s lambda[b,s], geff[b,e] = gate mass routed to expert e.

Sharding: data-parallel over batch B=32 across 8 cores (4 samples/core);
the [6,4,C,C,4,4] expert weights are replicated in bf16 (packed host-side
into the exact SBUF image, scaled by 0.0625 to fold the idwt/broadcast
constants). x streams in 1MB sub-tiles with partial block-sum reduces;
weights stream per-band so matmuls start before the full load; the final
fused pass streams per sub-tile (DVE + GpSimd) so y DMA-out overlaps.
"""

import numpy as np

import concourse.bacc as bacc
import concourse.mybir as mybir
import concourse.tile as tile

N_CORES = 8
B, C, S = 32, 64, 64
BL = B // N_CORES          # samples per core = 4
NE = 6                     # live experts
NCH = 4                    # x sub-tiles per row-tile
f32 = mybir.dt.float32
bf16 = mybir.dt.bfloat16
ALU = mybir.AluOpType


def _build_nc():
    nc = bacc.Bacc()
    xw = nc.declare_dram_parameter("xw", [2, 128, 4096], f32, isOutput=False)
    wt = nc.declare_dram_parameter("wt", [4, 128, 3072], bf16, isOutput=False)
    gt = nc.declare_dram_parameter("gt", [2, 128, 8], f32, isOutput=False)
    yw = nc.declare_dram_parameter("yw", [2, 128, 4096], f32, isOutput=True)

    with tile.TileContext(nc) as tc:
        with (
            tc.tile_pool(name="xp", bufs=8) as xp,
            tc.tile_pool(name="yp", bufs=8) as yp,
            tc.tile_pool(name="wp", bufs=4) as wp,
            tc.tile_pool(name="sp", bufs=2) as sp,
            tc.tile_pool(name="cp", bufs=3) as cp,
            tc.tile_pool(name="tp", bufs=8) as ttp,
            tc.tile_pool(name="ps", bufs=4, space="PSUM") as psp,
        ):
            gt_s, xs = [], [[], []]
            for rt in range(2):
                g = sp.tile([128, 8], f32, tag="gt", name=f"g{rt}")
                nc.sync.dma_start(out=g[:, :], in_=gt[rt, :, :])
                gt_s.append(g)

            wt_b = []
            for band in range(4):
                w = wp.tile([128, 3072], bf16, tag="wt", name=f"w{band}")
                wt_b.append(w)

            # interleave x sub-tile and weight-band DMA issue; x leads
            order = [("x", 0, 0), ("x", 0, 1), ("x", 0, 2), ("x", 0, 3),
                     ("x", 1, 0), ("x", 1, 1), ("x", 1, 2), ("x", 1, 3),
                     ("w", 0, 0), ("w", 1, 0), ("w", 2, 0), ("w", 3, 0)]
            for kind, a, c in order:
                if kind == "x":
                    xt = xp.tile([128, 1024], f32, tag="xs", name=f"x{a}{c}")
                    nc.sync.dma_start(out=xt[:, :], in_=xw[a, :, 1024 * c:1024 * (c + 1)])
                    xs[a].append(xt)
                else:
                    nc.sync.dma_start(out=wt_b[a][:, :], in_=wt[a, :, :])

            cc = cp.tile([128, 768], bf16, tag="cc", name="cc")
            coef, s8 = [], []
            for rt in range(2):
                # 8x8 block sums, streamed per sub-tile (w-dir), then h-dir
                r1 = sp.tile([128, 512], f32, tag="r1", name=f"r1{rt}")
                for c in range(NCH):
                    nc.vector.tensor_reduce(
                        out=r1[:, 128 * c:128 * (c + 1)].rearrange("p (h v) -> p h v", h=16),
                        in_=xs[rt][c][:, :].rearrange("p (h v w) -> p h v w", h=16, v=8, w=8),
                        axis=mybir.AxisListType.X, op=ALU.add,
                    )
                s8t = sp.tile([128, 64], f32, tag="s8", name=f"s8{rt}")
                nc.vector.tensor_reduce(
                    out=s8t[:, :].rearrange("p (u v) -> p u v", u=8),
                    in_=r1[:, :].rearrange("p (u dh v) -> p u v dh", u=8, dh=8, v=8),
                    axis=mybir.AxisListType.X, op=ALU.add,
                )
                s8.append(s8t)

                # level-4 Haar analysis on 0.0625*s8 (scale folds ll3 + one dwt level)
                sc = sp.tile([128, 64], f32, tag="sc", name=f"sc{rt}")
                nc.vector.tensor_scalar(out=sc[:, :], in0=s8t[:, :],
                                        scalar1=0.0625, scalar2=None, op0=ALU.mult)
                # merged quad combines: {t1,t2} = even+odd, {t3,t4} = even-odd
                # even = {a00,a10}: offsets {0,8}; odd = {a01,a11}: offsets {1,9}
                ev = sc[:, 0:64].rearrange("p (x i y j) -> p i j x y",
                                           x=4, i=2, y=4, j=2)[:, :, 0]
                od = sc[:, 0:64].rearrange("p (x i y j) -> p i j x y",
                                           x=4, i=2, y=4, j=2)[:, :, 1]
                tt = ttp.tile([128, 64], f32, tag="tt", name=f"tt{rt}")
                t2v = lambda o: tt[:, 32 * o:32 * (o + 1)].rearrange(
                    "p (g x y) -> p g x y", g=2, x=4, y=4)
                nc.vector.tensor_add(t2v(0), ev, od)   # t1(a00+a01), t2(a10+a11)
                nc.vector.tensor_sub(t2v(1), ev, od)   # t3, t4
                cf = sp.tile([128, 64], f32, tag="coef", name=f"cf{rt}")
                pick = lambda t, o: t[:, :].rearrange(
                    "p (g h m) -> p h g m", g=2, h=2, m=16)[:, o]
                nc.vector.tensor_add(pick(cf, 0), pick(tt, 0), pick(tt, 1))  # ll, hl
                nc.vector.tensor_sub(pick(cf, 1), pick(tt, 0), pick(tt, 1))  # lh, hh
                coef.append(cf)

            # gate-scaled channel-transposed coefficients:
            # cc[el*64+i, ch*256 + b*64 + bm], one op per (rt, bh, el):
            # out spans the 3 ch blocks; in0 broadcasts cf over ch; the gate
            # operand walks gt cols 1+el, 3+el, 5+el (stride 2) per ch block.
            for rt in range(2):
                cf = coef[rt]
                for bh in range(2):
                    b = rt * 2 + bh
                    for el in range(2):
                        nc.vector.tensor_tensor(
                            out=cc[el * 64:(el + 1) * 64, :]
                                .rearrange("p (ch bb m) -> p ch bb m", ch=3, bb=4, m=64)[:, :, b],
                            in0=cf[bh * 64:(bh + 1) * 64, :]
                                .rearrange("p (o m) -> p o m", o=1)
                                .broadcast_to([64, 3, 64]),
                            in1=gt_s[rt][bh * 64:(bh + 1) * 64, 1 + el:6 + el:2]
                                .rearrange("p (c o) -> p c o", c=3, o=1)
                                .broadcast_to([64, 3, 64]),
                            op=ALU.mult,
                        )

            # per-mode channel mixing, gate-combined via K=(e,i) accumulation
            pb = [psp.tile([64, 64], f32, tag="pb", name=f"pb{i}") for i in range(4)]
            for band in range(4):
                for mode in range(16):
                    for ch in range(3):
                        nc.tensor.matmul(
                            out=pb[band][:, mode * 4:(mode + 1) * 4],
                            lhsT=wt_b[band][:, (mode * 3 + ch) * 64:(mode * 3 + ch + 1) * 64],
                            rhs=cc[:, ch * 256 + band * 16 + mode:ch * 256 + band * 16 + mode + 193:64],
                            start=(ch == 0), stop=(ch == 2),
                        )

            # level-4 Haar synthesis (scale folded into weights) scattered per-sample
            sb1 = ttp.tile([64, 64], f32, tag="sb1")
            sb3 = ttp.tile([64, 64], f32, tag="sb3")
            nc.vector.tensor_copy(sb1[:, :], pb[1][:, :])
            nc.vector.tensor_copy(sb3[:, :], pb[3][:, :])
            u13 = ttp.tile([64, 128], f32, tag="u13")
            u24 = ttp.tile([64, 128], f32, tag="u24")
            nc.vector.tensor_add(u13[:, 0:64], pb[0][:, :], sb1[:, :])
            nc.vector.tensor_add(u24[:, 0:64], pb[2][:, :], sb3[:, :])
            nc.vector.tensor_sub(u13[:, 64:128], pb[0][:, :], sb1[:, :])
            nc.vector.tensor_sub(u24[:, 64:128], pb[2][:, :], sb3[:, :])

            adj_hs = []
            for rt in range(2):
                at = sp.tile([128, 64], f32, tag="adjT", name=f"at{rt}")
                for bh in range(2):
                    b = rt * 2 + bh
                    ov = at[bh * 64:(bh + 1) * 64, :].rearrange(
                        "p (x di y dj) -> p dj di x y", x=4, di=2, y=4, dj=2)
                    sv = lambda t: t[:, :].rearrange(
                        "p (k x y bb) -> p bb k x y", k=2, x=4, y=4, bb=4)[:, b]
                    nc.vector.tensor_add(ov[:, 0], sv(u13), sv(u24))
                    nc.vector.tensor_sub(ov[:, 1], sv(u13), sv(u24))
                # adjF = adjT + (-G/64) * s8   (gt col 7 = -G/64)
                adjF = sp.tile([128, 64], f32, tag="adjF", name=f"af{rt}")
                nc.vector.scalar_tensor_tensor(
                    out=adjF[:, :], in0=s8[rt][:, :], scalar=gt_s[rt][:, 7:8],
                    in1=at[:, :], op0=ALU.mult, op1=ALU.add,
                )
                # expand over h-rep: adj_h[p, u*64 + dh*8 + v] = adjF[p, u*8+v]
                adj_h = sp.tile([128, 512], f32, tag="adjh", name=f"ah{rt}")
                nc.vector.tensor_copy(
                    out=adj_h[:, :].rearrange("p (u dh v) -> p u dh v", u=8, dh=8, v=8),
                    in_=adjF[:, :].rearrange("p (u o v) -> p u o v", u=8, o=1, v=8)
                        .broadcast_to([128, 8, 8, 8]),
                )
                adj_hs.append(adj_h)

            # y = G*x + rep8(adjF), one fused DVE pass per sub-tile, stores stream out
            for rt in range(2):
                for c in range(NCH):
                    ys = yp.tile([128, 1024], f32, tag="ys", name=f"y{rt}{c}")
                    nc.vector.scalar_tensor_tensor(
                        out=ys[:, :].rearrange("p (hv w) -> p hv w", w=8),
                        in0=xs[rt][c][:, :].rearrange("p (hv w) -> p hv w", w=8),
                        scalar=gt_s[rt][:, 0:1],
                        in1=adj_hs[rt][:, 128 * c:128 * (c + 1)]
                            .rearrange("p (hv o) -> p hv o", o=1)
                            .broadcast_to([128, 128, 8]),
                        op0=ALU.mult, op1=ALU.add,
                    )
                    nc.sync.dma_start(out=yw[rt, :, 1024 * c:1024 * (c + 1)], in_=ys[:, :])
    nc.compile()
    return nc


_NC = None


def _get_nc():
    global _NC
    if _NC is None:
        _NC = _build_nc()
    return _NC


def _pack_weights(WL, WH):
    # Wall[band, e, i, o, x, y]; band 0 = WL, bands 1..3 = WH[:, k-1]
    Wall = np.empty((4, NE, C, C, 4, 4), np.float32)
    Wall[0] = WL[:NE]
    for k in range(3):
        Wall[k + 1] = WH[:NE, k]
    Wall *= 0.0625  # folds idwt 0.5 and rep8 0.125 scales
    # wt[band][el*64+i, ((x*4+y)*3 + ch)*64 + o]
    W6 = Wall.reshape(4, 3, 2, C, C, 4, 4)            # band, ch, el, i, o, x, y
    T = W6.transpose(0, 2, 3, 5, 6, 1, 4)             # band, el, i, x, y, ch, o
    import ml_dtypes
    return np.ascontiguousarray(T.reshape(4, 128, 3072)).astype(ml_dtypes.bfloat16)


def _pack_gates(lambda_):
    lam = lambda_.reshape(B, 8).astype(np.float32)
    G = lam.sum(1)
    geff = lam[:, :6].copy()
    geff[:, 4] += lam[:, 6]
    geff[:, 5] += lam[:, 7]
    gt = np.zeros((B, 8), np.float32)
    gt[:, 0] = G
    gt[:, 1:7] = geff
    gt[:, 7] = -G / 64.0
    return gt


def kernel(x, lambda_, WL, WH):
    from concourse.bass_utils import run_bass_kernel_spmd

    nc = _get_nc()
    wt = _pack_weights(np.asarray(WL, np.float32), np.asarray(WH, np.float32))
    gt = _pack_gates(np.asarray(lambda_, np.float32))
    x = np.ascontiguousarray(np.asarray(x, np.float32))

    in_maps = []
    for k in range(N_CORES):
        xl = x[k * BL:(k + 1) * BL].reshape(2, 128, 4096)
        gl = np.repeat(gt[k * BL:(k + 1) * BL], C, axis=0).reshape(2, 128, 8)
        in_maps.append({"xw": np.ascontiguousarray(xl),
                        "wt": wt,
                        "gt": np.ascontiguousarray(gl)})

    res = run_bass_kernel_spmd(nc, in_maps, list(range(N_CORES)))
    out = np.empty((B, C, S, S), np.float32)
    for k in range(N_CORES):
        out[k * BL:(k + 1) * BL] = res.results[k]["yw"].reshape(BL, C, S, S)
    return out



# revision 8
# speedup vs baseline: 1.1268x; 1.1268x over previous
"""Bass/Trainium2 kernel for nn_Expert_WNO2d (8-expert gated WaveConv2d mixture).

Math: the reference is linear in x. Every expert passes the fine Haar detail
levels (1..3) through unchanged and only channel-mixes the coarsest (level-4)
approximation + detail coefficients. With gate slots s weighting experts
PERM = (0,1,2,3,4,5,4,5), the output collapses to

    y[b] = G[b] * x[b] + rep8( adj[b] )                      (rep8 = 8x8 block broadcast)
    adj[b] = idwt4( sum_e geff[b,e] * (W_e . c4[b]) )*0.125*0.5 - (G[b]/64) * s8[b]

where s8 = 8x8 block sums of x, c4 = level-4 Haar coefficients (from s8),
G[b] = sum_s lambda[b,s], geff[b,e] = gate mass routed to expert e.

Sharding: data-parallel over batch B=32 across 8 cores (4 samples/core).

I/O precision (tolerance is rel 2e-2; this lands ~4e-3): x and y travel as
bf16 (host cast), expert weights as fp8 e4m3 scaled by 2^12 into fp8's
normal range; ALL descale factors (2^-16 fp8 descale * 0.0625 synthesis
fold) are pre-multiplied into the per-sample gate vector on the host, so
the device applies no descale ops at all.

Schedule: x streams in 4 sub-tiles per row-tile (rt); 8x8 block sums are a
dense bf16 pairwise tree over h (2x DVE) + one w-direction tensor_reduce.
Matmuls pack mode pairs into 128-col fp8 lhsT (FWL) and split N over
row-tiles so rt0's matmul/synthesis/y run under rt1's DMA shadow. A junk-MM
warmup block keeps the PE HAM clock at 8/8. cc build and one y sub-tile per
rt run on GPSIMD; x/y DMAs issue from sync, weights from scalar (both HWDGE).
"""

import numpy as np

import concourse.bacc as bacc
import concourse.mybir as mybir
import concourse.tile as tile

N_CORES = 8
B, C, S = 32, 64, 64
BL = B // N_CORES          # samples per core = 4
NE = 6                     # live experts
NCH = 4                    # x sub-tiles per row-tile
f32 = mybir.dt.float32
bf16 = mybir.dt.bfloat16
fp8 = mybir.dt.float8e4
ALU = mybir.AluOpType
AX = mybir.AxisListType

W_SCALE = 4096.0           # host weight scale into fp8 normal range
GATE_DESCALE = 1.0 / (W_SCALE * 16.0 * 16.0 * 4.0)  # 2^-20: fp8 descale + 0.0625 fold


def _build_nc():
    nc = bacc.Bacc()
    xw = nc.declare_dram_parameter("xw", [2, 128, 4096], bf16, isOutput=False)
    wt = nc.declare_dram_parameter("wt", [4, 128, 3072], fp8, isOutput=False)
    gt = nc.declare_dram_parameter("gt", [128, 16], f32, isOutput=False)
    yw = nc.declare_dram_parameter("yw", [2, 128, 4096], bf16, isOutput=True)

    with tile.TileContext(nc) as tc:
        with (
            tc.tile_pool(name="xp", bufs=8) as xp,
            tc.tile_pool(name="yp", bufs=8) as yp,
            tc.tile_pool(name="wp", bufs=4) as wp,
            tc.tile_pool(name="sp", bufs=2) as sp,
            tc.tile_pool(name="tp", bufs=4) as ttp,
            tc.tile_pool(name="ps", bufs=1, space="PSUM") as psp,
        ):
            # single PSUM tile, cols = band*64 + mp*8 + j*4 + b
            pq = psp.tile([128, 256], f32, tag="pq", name="pq")

            # ---- PE warmup: junk matmuls with no data deps keep HAM at 8/8
            # (write into pq; real MMs later restart the accumulation group)
            junk = sp.tile([128, 32], bf16, tag="junk", name="junk")
            nc.gpsimd.memset(junk[:, :], 0.0)
            for i in range(72):
                nc.tensor.matmul(
                    out=pq[0:32, 0:1], lhsT=junk[:, 0:32], rhs=junk[:, 0:1],
                    start=True, stop=True,
                )

            # ---- DMA in: gates + weights on scalar engine, x on sync engine
            gt_s = sp.tile([128, 16], f32, tag="gt", name="gt")
            nc.scalar.dma_start(out=gt_s[:, :], in_=gt[:, :])

            xs = [[], []]
            for rt in range(2):
                for c in range(NCH):
                    xt = xp.tile([128, 1024], bf16, tag="xs", name=f"x{rt}{c}")
                    nc.sync.dma_start(out=xt[:, :], in_=xw[rt, :, 1024 * c:1024 * (c + 1)])
                    xs[rt].append(xt)

            wt_b = []
            for band in range(4):
                w = wp.tile([128, 3072], fp8, tag="wt", name=f"w{band}")
                nc.scalar.dma_start(out=w[:, :], in_=wt[band, :, :])
                wt_b.append(w)

            # ---- per-rt coefficient chain ------------------------------
            # x sub-tile cols = (h=16, v=8, w=8). Dense bf16 pairwise tree
            # over h (pairs are 512/256/128 apart -> unit-stride inner dim),
            # then one 1x tensor_reduce over w.
            s8 = []
            cf_t = []
            cc = sp.tile([128, 768], bf16, tag="cc", name="cc")

            def coeff_chain(rt):
                h3 = ttp.tile([128, 512], bf16, tag="h3", name=f"h3{rt}")
                for c in range(NCH):
                    a = xs[rt][c]
                    t1 = ttp.tile([128, 512], bf16, tag="t1", name=f"t1{rt}{c}")
                    v = lambda t, n: t[:, :].rearrange("p (hb q) -> p hb q", hb=2, q=n)
                    av = a[:, :].rearrange("p (hb h2 q) -> p hb h2 q", hb=2, h2=2, q=256)
                    nc.vector.tensor_add(v(t1, 256), av[:, :, 0], av[:, :, 1])
                    t2 = ttp.tile([128, 256], bf16, tag="t2", name=f"t2{rt}{c}")
                    t1v = t1[:, :].rearrange("p (hb h2 q) -> p hb h2 q", hb=2, h2=2, q=128)
                    nc.vector.tensor_add(v(t2, 128), t1v[:, :, 0], t1v[:, :, 1])
                    t2v = t2[:, :].rearrange("p (hb h2 q) -> p hb h2 q", hb=2, h2=2, q=64)
                    nc.vector.tensor_add(
                        h3[:, 128 * c:128 * (c + 1)].rearrange("p (hb q) -> p hb q", hb=2, q=64),
                        t2v[:, :, 0], t2v[:, :, 1],
                    )
                # w-direction reduce: h3 cols = (u=8 h-blocks, v=8, w=8)
                s8t = sp.tile([128, 64], f32, tag="s8", name=f"s8{rt}")
                nc.vector.tensor_reduce(
                    out=s8t[:, :].rearrange("p (u v) -> p u v", u=8),
                    in_=h3[:, :].rearrange("p (u v w) -> p u v w", u=8, v=8, w=8),
                    axis=AX.X, op=ALU.add,
                )
                s8.append(s8t)

                # level-4 Haar analysis directly on s8 (scales folded into gates)
                ev = s8t[:, 0:64].rearrange("p (x i y j) -> p i j x y",
                                            x=4, i=2, y=4, j=2)[:, :, 0]
                od = s8t[:, 0:64].rearrange("p (x i y j) -> p i j x y",
                                            x=4, i=2, y=4, j=2)[:, :, 1]
                tt = ttp.tile([128, 64], f32, tag="tt", name=f"tt{rt}")
                t2v2 = lambda o: tt[:, 32 * o:32 * (o + 1)].rearrange(
                    "p (g x y) -> p g x y", g=2, x=4, y=4)
                nc.vector.tensor_add(t2v2(0), ev, od)
                nc.vector.tensor_sub(t2v2(1), ev, od)
                cf = sp.tile([128, 64], f32, tag="coef", name=f"cf{rt}")
                pick = lambda t, o: t[:, :].rearrange(
                    "p (g h m) -> p h g m", g=2, h=2, m=16)[:, o]
                nc.vector.tensor_add(pick(cf, 0), pick(tt, 0), pick(tt, 1))
                nc.vector.tensor_sub(pick(cf, 1), pick(tt, 0), pick(tt, 1))
                cf_t.append(cf)

                # gate-scaled coefficients, cc[el*64+i, ch*256 + q*4 + b]
                # (q = band*16+mode); gates already fold all descales. GPSIMD.
                ccv = cc[:, :].rearrange("p (ch q b) -> p b ch q", ch=3, q=64, b=4)
                for bh in range(2):
                    b = rt * 2 + bh
                    for el in range(2):
                        nc.gpsimd.tensor_tensor(
                            out=ccv[el * 64:(el + 1) * 64, b],
                            in0=cf[bh * 64:(bh + 1) * 64, :]
                                .rearrange("p (o q) -> p o q", o=1)
                                .broadcast_to([64, 3, 64]),
                            in1=gt_s[bh * 64:(bh + 1) * 64, 8 * rt + 1 + el:8 * rt + 6 + el:2]
                                .rearrange("p (c o) -> p c o", c=3, o=1)
                                .broadcast_to([64, 3, 64]),
                            op=ALU.mult,
                        )

            # ---- matmuls: lhsT[128=(el,i), 128=(j,o)] per (band, mp, ch),
            # FWL-eligible fp8; rhs N=4 = (j=2, b=2) per rt so rt0's block
            # runs during rt1's x DMA. K-accum over ch (expert pairs).
            def mm_block(rt):
                pbv = pq[:, :].rearrange("p (band mp j b) -> p band mp j b",
                                         band=4, mp=8, j=2, b=4)
                ccv = cc[:, :].rearrange("p (ch bm j b) -> p ch bm j b",
                                         ch=3, bm=32, j=2, b=4)
                for band in range(4):
                    for mp in range(8):
                        for ch in range(3):
                            nc.tensor.matmul(
                                out=pbv[:, band, mp, :, 2 * rt:2 * rt + 2],
                                lhsT=wt_b[band][:, (mp * 3 + ch) * 128:(mp * 3 + ch + 1) * 128],
                                rhs=ccv[:, ch, band * 8 + mp, :, 2 * rt:2 * rt + 2],
                                start=(ch == 0), stop=(ch == 2),
                            )

            # ---- synthesis + fused output pass --------------------------
            u13s, u24s = [], []
            for rt in range(2):
                u13 = ttp.tile([64, 128], f32, tag="u13", name=f"u13{rt}")
                u24 = ttp.tile([64, 128], f32, tag="u24", name=f"u24{rt}")
                u13s.append(u13)
                u24s.append(u24)

            def synth(rt):
                # copy this rt's PSUM slice to SBUF (walrus: DVE may read at
                # most one PSUM operand), then u13 = pb0 +/- pb1, u24 = pb2
                # +/- pb3 with cols k*64 + m*4 + b; valid pq quadrants are
                # rows j*64+o at cols mp*8 + j*4 + b.
                pbs = sp.tile([128, 128], f32, tag="pbs", name=f"pbs{rt}")
                pqv = pq[:, :].rearrange("p (band mp j b) -> p j band mp b",
                                         band=4, mp=8, j=2, b=4)
                psv = pbs[:, :].rearrange("p (band mp j b) -> p j band mp b",
                                          band=4, mp=8, j=2, b=2)
                for j in range(2):
                    nc.vector.tensor_copy(out=psv[:, j],
                                          in_=pqv[:, j, :, :, 2 * rt:2 * rt + 2])
                u13, u24 = u13s[rt], u24s[rt]
                for (u, lo, hi) in ((u13, 0, 1), (u24, 2, 3)):
                    uv = u[:, :].rearrange("p (k mp j b) -> p k j mp b",
                                           k=2, mp=8, j=2, b=4)
                    for j in range(2):
                        pv = lambda band: pbs[j * 64:(j + 1) * 64, :].rearrange(
                            "p (bd mp j2 b) -> p bd j2 mp b", bd=4, mp=8, j2=2, b=2)[:, band, j]
                        nc.vector.tensor_add(uv[:, 0, j, :, 2 * rt:2 * rt + 2], pv(lo), pv(hi))
                        nc.vector.tensor_sub(uv[:, 1, j, :, 2 * rt:2 * rt + 2], pv(lo), pv(hi))
                # idwt level-4 scatter + pass-through correction
                at = sp.tile([128, 64], f32, tag="adjT", name=f"at{rt}")
                for bh in range(2):
                    b = rt * 2 + bh
                    ov = at[bh * 64:(bh + 1) * 64, :].rearrange(
                        "p (x di y dj) -> p dj di x y", x=4, di=2, y=4, dj=2)
                    sv = lambda t: t[:, :].rearrange(
                        "p (k x y bb) -> p bb k x y", k=2, x=4, y=4, bb=4)[:, b]
                    nc.vector.tensor_add(ov[:, 0], sv(u13), sv(u24))
                    nc.vector.tensor_sub(ov[:, 1], sv(u13), sv(u24))
                adjF = sp.tile([128, 64], f32, tag="adjF", name=f"af{rt}")
                nc.vector.scalar_tensor_tensor(
                    out=adjF[:, :], in0=s8[rt][:, :], scalar=gt_s[:, 8 * rt + 7:8 * rt + 8],
                    in1=at[:, :], op0=ALU.mult, op1=ALU.add,
                )
                adj_h = sp.tile([128, 512], bf16, tag="adjh", name=f"ah{rt}")
                nc.vector.tensor_copy(
                    out=adj_h[:, :].rearrange("p (u dh v) -> p u dh v", u=8, dh=8, v=8),
                    in_=adjF[:, :].rearrange("p (u o v) -> p u o v", u=8, o=1, v=8)
                        .broadcast_to([128, 8, 8, 8]),
                )
                return adj_h

            def y_pass(rt, adj_h):
                # y = G*x + rep8(adj)
                for c in range(NCH):
                    ys = yp.tile([128, 1024], bf16, tag="ys", name=f"y{rt}{c}")
                    eng = nc.vector
                    eng.scalar_tensor_tensor(
                        out=ys[:, :].rearrange("p (hv w) -> p hv w", w=8),
                        in0=xs[rt][c][:, :].rearrange("p (hv w) -> p hv w", w=8),
                        scalar=gt_s[:, 8 * rt:8 * rt + 1],
                        in1=adj_h[:, 128 * c:128 * (c + 1)]
                            .rearrange("p (hv o) -> p hv o", o=1)
                            .broadcast_to([128, 128, 8]),
                        op0=ALU.mult, op1=ALU.add,
                    )
                    nc.sync.dma_start(out=yw[rt, :, 1024 * c:1024 * (c + 1)], in_=ys[:, :])

            coeff_chain(0)
            mm_block(0)
            coeff_chain(1)
            mm_block(1)
            adj0 = synth(0)
            y_pass(0, adj0)
            adj1 = synth(1)
            y_pass(1, adj1)
    nc.compile()
    return nc


_NC = None


def _get_nc():
    global _NC
    if _NC is None:
        _NC = _build_nc()
    return _NC


def _pack_weights(WL, WH):
    import ml_dtypes
    # Wall[band, e, i, o, x, y]; band 0 = WL, bands 1..3 = WH[:, k-1]
    Wall = np.empty((4, NE, C, C, 4, 4), np.float32)
    Wall[0] = WL[:NE]
    for k in range(3):
        Wall[k + 1] = WH[:NE, k]
    Wall *= W_SCALE
    # wt[band][el*64+i, mp*384 + ch*128 + j*64 + o], e = ch*2+el, m = 2*mp+j
    W7 = Wall.reshape(4, 3, 2, C, C, 4, 4)            # band, ch, el, i, o, x, y
    T = W7.transpose(0, 2, 3, 5, 6, 1, 4)             # band, el, i, x, y, ch, o
    T = T.reshape(4, 2, C, 8, 2, 3, C)                # band, el, i, mp, j, ch, o
    T = T.transpose(0, 1, 2, 3, 5, 4, 6)              # band, el, i, mp, ch, j, o
    return np.ascontiguousarray(T.reshape(4, 128, 3072)).astype(ml_dtypes.float8_e4m3)


def _pack_gates(lambda_):
    lam = lambda_.reshape(B, 8).astype(np.float32)
    G = lam.sum(1)
    geff = lam[:, :6].copy()
    geff[:, 4] += lam[:, 6]
    geff[:, 5] += lam[:, 7]
    gtv = np.zeros((B, 8), np.float32)
    gtv[:, 0] = G
    gtv[:, 1:7] = geff * GATE_DESCALE
    gtv[:, 7] = -G / 64.0
    return gtv


def _build_in_maps(x, lambda_, WL, WH):
    import ml_dtypes
    wtp = _pack_weights(np.asarray(WL, np.float32), np.asarray(WH, np.float32))
    gtv = _pack_gates(np.asarray(lambda_, np.float32))
    xb = np.asarray(x, np.float32).astype(ml_dtypes.bfloat16)

    in_maps = []
    for k in range(N_CORES):
        xl = xb[k * BL:(k + 1) * BL].reshape(2, 128, 4096)
        # gt[bh*64+i, rt*8 + col] = gtv[k*BL + rt*2 + bh, col]
        gl = np.empty((128, 16), np.float32)
        for rt in range(2):
            for bh in range(2):
                gl[bh * 64:(bh + 1) * 64, rt * 8:(rt + 1) * 8] = gtv[k * BL + rt * 2 + bh]
        in_maps.append({"xw": np.ascontiguousarray(xl),
                        "wt": wtp,
                        "gt": gl})
    return in_maps


def kernel(x, lambda_, WL, WH):
    from concourse.bass_utils import run_bass_kernel_spmd

    nc = _get_nc()
    in_maps = _build_in_maps(x, lambda_, WL, WH)
    res = run_bass_kernel_spmd(nc, in_maps, list(range(N_CORES)))
    out = np.empty((B, C, S, S), np.float32)
    for k in range(N_CORES):
        out[k * BL:(k + 1) * BL] = np.asarray(
            res.results[k]["yw"], dtype=np.float32).reshape(BL, C, S, S)
    return out


# revision 9
# speedup vs baseline: 1.1868x; 1.0533x over previous
"""Bass/Trainium2 kernel for nn_Expert_WNO2d (8-expert gated WaveConv2d mixture).

Math: the reference is linear in x. Every expert passes the fine Haar detail
levels (1..3) through unchanged and only channel-mixes the coarsest (level-4)
approximation + detail coefficients. With gate slots s weighting experts
PERM = (0,1,2,3,4,5,4,5), the output collapses to

    y[b] = G[b] * x[b] + rep8( adj[b] )                      (rep8 = 8x8 block broadcast)
    adj[b] = idwt4( sum_e geff[b,e] * (W_e . c4[b]) )*0.0625 - (G[b]/64) * s8[b]

where s8 = 8x8 block sums of x, c4 = level-4 Haar coefficients (from s8),
G[b] = sum_s lambda[b,s], geff[b,e] = gate mass routed to expert e.

Sharding: data-parallel over batch B=32 across 8 cores (4 samples/core).

I/O precision (tolerance is rel 2e-2; this lands ~5e-3): x and y travel as
bf16 (host cast), expert weights as fp8 e4m3 scaled by 2^12 into fp8's
normal range; ALL descale factors (2^-16 fp8 descale * 0.0625 synthesis
fold) are pre-multiplied into the per-sample gate vector on the host, so
the device applies no descale ops at all.

Schedule: all HWDGE DMA data drains through one FIFO ring, so every DMA is
issued from the sync engine in exact consumption order (x-rt0, W bands,
x-rt1, y stores). 8x8 block sums use a dense bf16 pairwise tree over h
(2x DVE) + one w-direction tensor_reduce. Matmuls pack mode pairs into
128-col fp8 lhsT (FWL) with N split by row-tile so rt0's matmul/synthesis/y
run under rt1's DMA shadow; a junk-MM warmup block holds the PE HAM clock
at 8/8. The scalar engine computes G*x per chunk as data lands (off the
critical path); DVE adds the broadcast adj; GPSIMD takes part of cc.
"""

import numpy as np

import concourse.bacc as bacc
import concourse.mybir as mybir
import concourse.tile as tile

N_CORES = 8
B, C, S = 32, 64, 64
BL = B // N_CORES          # samples per core = 4
NE = 6                     # live experts
NCH = 2                    # x chunks per row-tile, [128, 2048] each
f32 = mybir.dt.float32
bf16 = mybir.dt.bfloat16
fp8 = mybir.dt.float8e4
ALU = mybir.AluOpType
AX = mybir.AxisListType
AF = mybir.ActivationFunctionType

W_SCALE = 4096.0           # host weight scale into fp8 normal range
GATE_DESCALE = 1.0 / (W_SCALE * 16.0 * 16.0 * 4.0)  # 2^-20: fp8 descale + 0.0625 fold


def _build_nc():
    nc = bacc.Bacc()
    xw = nc.declare_dram_parameter("xw", [2, 128, 4096], bf16, isOutput=False)
    wt = nc.declare_dram_parameter("wt", [4, 128, 3072], fp8, isOutput=False)
    gt = nc.declare_dram_parameter("gt", [128, 16], f32, isOutput=False)
    yw = nc.declare_dram_parameter("yw", [2, 128, 4096], bf16, isOutput=True)

    with tile.TileContext(nc) as tc:
        with (
            tc.tile_pool(name="xp", bufs=4) as xp,
            tc.tile_pool(name="yp", bufs=4) as yp,
            tc.tile_pool(name="wp", bufs=4) as wp,
            tc.tile_pool(name="sp", bufs=2) as sp,
            tc.tile_pool(name="tp", bufs=4) as ttp,
            tc.tile_pool(name="ps", bufs=1, space="PSUM") as psp,
        ):
            # single PSUM tile, cols = band*64 + mp*8 + j*4 + b
            pq = psp.tile([128, 256], f32, tag="pq", name="pq")

            # ---- PE warmup: junk matmuls with no data deps keep HAM at 8/8
            junk = sp.tile([128, 32], bf16, tag="junk", name="junk")
            nc.gpsimd.memset(junk[:, :], 0.0)
            for i in range(72):
                nc.tensor.matmul(
                    out=pq[0:32, 0:1], lhsT=junk[:, 0:32], rhs=junk[:, 0:1],
                    start=True, stop=True,
                )

            # ---- DMA in: single FIFO ring -> issue in consumption order
            gt_s = sp.tile([128, 16], f32, tag="gt", name="gt")
            nc.sync.dma_start(out=gt_s[:, :], in_=gt[:, :])

            xs = [[], []]
            wt_b = [wp.tile([128, 3072], fp8, tag="wt", name=f"w{band}")
                    for band in range(4)]

            def load_x(rt):
                for c in range(NCH):
                    xt = xp.tile([128, 2048], bf16, tag="xs", name=f"x{rt}{c}")
                    nc.sync.dma_start(out=xt[:, :], in_=xw[rt, :, 2048 * c:2048 * (c + 1)])
                    xs[rt].append(xt)

            load_x(0)
            for band in range(4):
                nc.sync.dma_start(out=wt_b[band][:, :], in_=wt[band, :, :])
            load_x(1)

            # ---- G*x on the scalar engine, per chunk as data lands
            ys_g = [[], []]
            for rt in range(2):
                for c in range(NCH):
                    yg = yp.tile([128, 2048], bf16, tag="ysg", name=f"yg{rt}{c}")
                    nc.scalar.activation(
                        out=yg[:, :], in_=xs[rt][c][:, :], func=AF.Copy,
                        scale=gt_s[:, 8 * rt:8 * rt + 1],
                    )
                    ys_g[rt].append(yg)

            # ---- per-rt coefficient chain ------------------------------
            # chunk cols = (h=32, v=8, w=8). Dense bf16 pairwise tree over
            # h within each 8-row block (unit-stride inner dims), then one
            # 1x tensor_reduce over w.
            s8 = []
            cc = sp.tile([128, 768], bf16, tag="cc", name="cc")

            def coeff_chain(rt):
                h3 = ttp.tile([128, 512], bf16, tag="h3", name=f"h3{rt}")
                for c in range(NCH):
                    a = xs[rt][c]
                    t1 = ttp.tile([128, 1024], bf16, tag="t1", name=f"t1{rt}{c}")
                    v = lambda t, n: t[:, :].rearrange("p (hb q) -> p hb q", hb=4, q=n)
                    av = a[:, :].rearrange("p (hb h2 q) -> p hb h2 q", hb=4, h2=2, q=256)
                    nc.vector.tensor_add(v(t1, 256), av[:, :, 0], av[:, :, 1])
                    t2 = ttp.tile([128, 512], bf16, tag="t2", name=f"t2{rt}{c}")
                    t1v = t1[:, :].rearrange("p (hb h2 q) -> p hb h2 q", hb=4, h2=2, q=128)
                    nc.vector.tensor_add(v(t2, 128), t1v[:, :, 0], t1v[:, :, 1])
                    t2v = t2[:, :].rearrange("p (hb h2 q) -> p hb h2 q", hb=4, h2=2, q=64)
                    nc.vector.tensor_add(
                        h3[:, 256 * c:256 * (c + 1)].rearrange("p (hb q) -> p hb q", hb=4, q=64),
                        t2v[:, :, 0], t2v[:, :, 1],
                    )
                # w-direction reduce: h3 cols = (u=8 h-blocks, v=8, w=8)
                s8t = sp.tile([128, 64], f32, tag="s8", name=f"s8{rt}")
                nc.vector.tensor_reduce(
                    out=s8t[:, :].rearrange("p (u v) -> p u v", u=8),
                    in_=h3[:, :].rearrange("p (u v w) -> p u v w", u=8, v=8, w=8),
                    axis=AX.X, op=ALU.add,
                )
                s8.append(s8t)

                # level-4 Haar analysis directly on s8 (scales folded into gates)
                ev = s8t[:, 0:64].rearrange("p (x i y j) -> p i j x y",
                                            x=4, i=2, y=4, j=2)[:, :, 0]
                od = s8t[:, 0:64].rearrange("p (x i y j) -> p i j x y",
                                            x=4, i=2, y=4, j=2)[:, :, 1]
                tt = ttp.tile([128, 64], f32, tag="tt", name=f"tt{rt}")
                t2v2 = lambda o: tt[:, 32 * o:32 * (o + 1)].rearrange(
                    "p (g x y) -> p g x y", g=2, x=4, y=4)
                nc.vector.tensor_add(t2v2(0), ev, od)
                nc.vector.tensor_sub(t2v2(1), ev, od)
                cf = sp.tile([128, 64], f32, tag="coef", name=f"cf{rt}")
                pick = lambda t, o: t[:, :].rearrange(
                    "p (g h m) -> p h g m", g=2, h=2, m=16)[:, o]
                nc.vector.tensor_add(pick(cf, 0), pick(tt, 0), pick(tt, 1))
                nc.vector.tensor_sub(pick(cf, 1), pick(tt, 0), pick(tt, 1))

                # gate-scaled coefficients, cc[el*64+i, ch*256 + q*4 + b]
                # (q = band*16+mode); gates fold all descales. Last op on
                # GPSIMD, rest on DVE.
                ccv = cc[:, :].rearrange("p (ch q b) -> p b ch q", ch=3, q=64, b=4)
                for bh in range(2):
                    b = rt * 2 + bh
                    for el in range(2):
                        eng = nc.gpsimd if (bh == 1 and el == 1) else nc.vector
                        eng.tensor_tensor(
                            out=ccv[el * 64:(el + 1) * 64, b],
                            in0=cf[bh * 64:(bh + 1) * 64, :]
                                .rearrange("p (o q) -> p o q", o=1)
                                .broadcast_to([64, 3, 64]),
                            in1=gt_s[bh * 64:(bh + 1) * 64, 8 * rt + 1 + el:8 * rt + 6 + el:2]
                                .rearrange("p (c o) -> p c o", c=3, o=1)
                                .broadcast_to([64, 3, 64]),
                            op=ALU.mult,
                        )

            # ---- matmuls: lhsT[128=(el,i), 128=(j,o)] per (band, mp, ch),
            # FWL-eligible fp8; rhs N=4 = (j=2, b=2) per rt so rt0's block
            # runs during rt1's x DMA. K-accum over ch (expert pairs).
            def mm_block(rt):
                pbv = pq[:, :].rearrange("p (band mp j b) -> p band mp j b",
                                         band=4, mp=8, j=2, b=4)
                ccv = cc[:, :].rearrange("p (ch bm j b) -> p ch bm j b",
                                         ch=3, bm=32, j=2, b=4)
                for band in range(4):
                    for mp in range(8):
                        for ch in range(3):
                            nc.tensor.matmul(
                                out=pbv[:, band, mp, :, 2 * rt:2 * rt + 2],
                                lhsT=wt_b[band][:, (mp * 3 + ch) * 128:(mp * 3 + ch + 1) * 128],
                                rhs=ccv[:, ch, band * 8 + mp, :, 2 * rt:2 * rt + 2],
                                start=(ch == 0), stop=(ch == 2),
                            )

            # ---- synthesis + fused output pass --------------------------
            u13s = [ttp.tile([64, 128], f32, tag="u13", name=f"u13{rt}") for rt in range(2)]
            u24s = [ttp.tile([64, 128], f32, tag="u24", name=f"u24{rt}") for rt in range(2)]

            def synth(rt):
                # copy this rt's PSUM slice to SBUF (walrus: DVE may read at
                # most one PSUM operand), then u13 = pb0 +/- pb1, u24 = pb2
                # +/- pb3 with cols k*64 + m*4 + b; valid pq quadrants are
                # rows j*64+o at cols mp*8 + j*4 + b.
                pbs = sp.tile([128, 128], f32, tag="pbs", name=f"pbs{rt}")
                pqv = pq[:, :].rearrange("p (band mp j b) -> p j band mp b",
                                         band=4, mp=8, j=2, b=4)
                psv = pbs[:, :].rearrange("p (band mp j b) -> p j band mp b",
                                          band=4, mp=8, j=2, b=2)
                for j in range(2):
                    nc.vector.tensor_copy(out=psv[:, j],
                                          in_=pqv[:, j, :, :, 2 * rt:2 * rt + 2])
                u13, u24 = u13s[rt], u24s[rt]
                for (u, lo, hi) in ((u13, 0, 1), (u24, 2, 3)):
                    uv = u[:, :].rearrange("p (k mp j b) -> p k j mp b",
                                           k=2, mp=8, j=2, b=4)
                    for j in range(2):
                        pv = lambda band: pbs[j * 64:(j + 1) * 64, :].rearrange(
                            "p (bd mp j2 b) -> p bd j2 mp b", bd=4, mp=8, j2=2, b=2)[:, band, j]
                        nc.vector.tensor_add(uv[:, 0, j, :, 2 * rt:2 * rt + 2], pv(lo), pv(hi))
                        nc.vector.tensor_sub(uv[:, 1, j, :, 2 * rt:2 * rt + 2], pv(lo), pv(hi))
                # idwt level-4 scatter + pass-through correction
                at = sp.tile([128, 64], f32, tag="adjT", name=f"at{rt}")
                for bh in range(2):
                    b = rt * 2 + bh
                    ov = at[bh * 64:(bh + 1) * 64, :].rearrange(
                        "p (x di y dj) -> p dj di x y", x=4, di=2, y=4, dj=2)
                    sv = lambda t: t[:, :].rearrange(
                        "p (k x y bb) -> p bb k x y", k=2, x=4, y=4, bb=4)[:, b]
                    nc.vector.tensor_add(ov[:, 0], sv(u13), sv(u24))
                    nc.vector.tensor_sub(ov[:, 1], sv(u13), sv(u24))
                adjF = sp.tile([128, 64], f32, tag="adjF", name=f"af{rt}")
                nc.vector.scalar_tensor_tensor(
                    out=adjF[:, :], in0=s8[rt][:, :], scalar=gt_s[:, 8 * rt + 7:8 * rt + 8],
                    in1=at[:, :], op0=ALU.mult, op1=ALU.add,
                )
                adj_h = sp.tile([128, 512], bf16, tag="adjh", name=f"ah{rt}")
                nc.vector.tensor_copy(
                    out=adj_h[:, :].rearrange("p (u dh v) -> p u dh v", u=8, dh=8, v=8),
                    in_=adjF[:, :].rearrange("p (u o v) -> p u o v", u=8, o=1, v=8)
                        .broadcast_to([128, 8, 8, 8]),
                )
                return adj_h

            def y_pass(rt, adj_h):
                # y = (G*x from scalar engine) + rep8(adj) on DVE
                for c in range(NCH):
                    ys = yp.tile([128, 2048], bf16, tag="ys", name=f"y{rt}{c}")
                    nc.vector.tensor_tensor(
                        out=ys[:, :].rearrange("p (hv w) -> p hv w", w=8),
                        in0=ys_g[rt][c][:, :].rearrange("p (hv w) -> p hv w", w=8),
                        in1=adj_h[:, 256 * c:256 * (c + 1)]
                            .rearrange("p (hv o) -> p hv o", o=1)
                            .broadcast_to([128, 256, 8]),
                        op=ALU.add,
                    )
                    nc.sync.dma_start(out=yw[rt, :, 2048 * c:2048 * (c + 1)], in_=ys[:, :])

            coeff_chain(0)
            mm_block(0)
            coeff_chain(1)
            mm_block(1)
            adj0 = synth(0)
            y_pass(0, adj0)
            adj1 = synth(1)
            y_pass(1, adj1)
    nc.compile()
    return nc


_NC = None


def _get_nc():
    global _NC
    if _NC is None:
        _NC = _build_nc()
    return _NC


def _pack_weights(WL, WH):
    import ml_dtypes
    # Wall[band, e, i, o, x, y]; band 0 = WL, bands 1..3 = WH[:, k-1]
    Wall = np.empty((4, NE, C, C, 4, 4), np.float32)
    Wall[0] = WL[:NE]
    for k in range(3):
        Wall[k + 1] = WH[:NE, k]
    Wall *= W_SCALE
    # wt[band][el*64+i, mp*384 + ch*128 + j*64 + o], e = ch*2+el, m = 2*mp+j
    W7 = Wall.reshape(4, 3, 2, C, C, 4, 4)            # band, ch, el, i, o, x, y
    T = W7.transpose(0, 2, 3, 5, 6, 1, 4)             # band, el, i, x, y, ch, o
    T = T.reshape(4, 2, C, 8, 2, 3, C)                # band, el, i, mp, j, ch, o
    T = T.transpose(0, 1, 2, 3, 5, 4, 6)              # band, el, i, mp, ch, j, o
    return np.ascontiguousarray(T.reshape(4, 128, 3072)).astype(ml_dtypes.float8_e4m3)


def _pack_gates(lambda_):
    lam = lambda_.reshape(B, 8).astype(np.float32)
    G = lam.sum(1)
    geff = lam[:, :6].copy()
    geff[:, 4] += lam[:, 6]
    geff[:, 5] += lam[:, 7]
    gtv = np.zeros((B, 8), np.float32)
    gtv[:, 0] = G
    gtv[:, 1:7] = geff * GATE_DESCALE
    gtv[:, 7] = -G / 64.0
    return gtv


def _build_in_maps(x, lambda_, WL, WH):
    import ml_dtypes
    wtp = _pack_weights(np.asarray(WL, np.float32), np.asarray(WH, np.float32))
    gtv = _pack_gates(np.asarray(lambda_, np.float32))
    xb = np.asarray(x, np.float32).astype(ml_dtypes.bfloat16)

    in_maps = []
    for k in range(N_CORES):
        xl = xb[k * BL:(k + 1) * BL].reshape(2, 128, 4096)
        # gt[bh*64+i, rt*8 + col] = gtv[k*BL + rt*2 + bh, col]
        gl = np.empty((128, 16), np.float32)
        for rt in range(2):
            for bh in range(2):
                gl[bh * 64:(bh + 1) * 64, rt * 8:(rt + 1) * 8] = gtv[k * BL + rt * 2 + bh]
        in_maps.append({"xw": np.ascontiguousarray(xl),
                        "wt": wtp,
                        "gt": gl})
    return in_maps


def kernel(x, lambda_, WL, WH):
    from concourse.bass_utils import run_bass_kernel_spmd

    nc = _get_nc()
    in_maps = _build_in_maps(x, lambda_, WL, WH)
    res = run_bass_kernel_spmd(nc, in_maps, list(range(N_CORES)))
    out = np.empty((B, C, S, S), np.float32)
    for k in range(N_CORES):
        out[k * BL:(k + 1) * BL] = np.asarray(
            res.results[k]["yw"], dtype=np.float32).reshape(BL, C, S, S)
    return out
